# revision 1
# baseline (speedup 1.0000x reference)
"""DilatedReparamConv (6 depthwise-conv branches + training-mode BN, summed)
as a Trainium2 Bass kernel.

Strategy:
  - Channel-parallel sharding: core i handles channels [32*i, 32*i+32) with the
    full batch, so BN batch-stats stay core-local (no collectives).
  - Depthwise conv runs on the TensorEngine as banded-matrix matmuls:
    stationary operand = per-(channel, kernel-column) banded matrix B with
    B[h, j] = V[h + j] (V = 223-long vertical kernel vector), moving operand =
    112 image rows x (4 images * 112 cols); horizontal taps are free-dim window
    shifts of the padded input; vertical accumulation happens in PSUM.
  - The skew (Toeplitz structure) of B is materialized by an overlapping-window
    DRAM->SBUF DMA from small per-channel V vectors (built on host for pass 1).
  - Pass 1 computes the 6 branch convs and per-channel sum / sum-of-squares
    (DVE reduce + ScalarE Square with accumulate). BN scales s_br and the total
    bias T are computed on-device; the 6 branches then merge into ONE 11x11
    kernel (V2 = sum_br s_br * V1_br), round-tripped through DRAM for the skew.
  - Pass 2 runs the single merged conv and adds T.
  - Host pre-flips image rows and stores V vertically reversed so every DMA
    stride is positive; the output comes out in natural row order.
"""
import numpy as np

import concourse.bass as bass
import concourse.tile as tile
from concourse import mybir

# ---------------------------------------------------------------------------
# Workaround for this walrus build: instructions only support a single
# semaphore wait in codegen ("Too many sync wait commands"), but Tile attaches
# as many waits as the dependence structure needs. Post-pass: hoist excess
# waits onto same-engine no-op instructions inserted right before the
# instruction (engine streams are in-order, so this is semantics-preserving).
_MAXW = 1


def _split_excess_waits(nc):
    for f in nc.m.functions:
        for b in f.blocks:
            new = []
            for inst in b.instructions:
                si = getattr(inst, "sync_info", None)
                waits = list(si.on_wait) if si is not None and si.on_wait else []
                if len(waits) > _MAXW:
                    extra = waits[: len(waits) - _MAXW]
                    del si.on_wait[: len(extra)]
                    for j in range(0, len(extra), _MAXW):
                        w_inst = mybir.InstDrain(
                            name=f"WSPLIT-{nc.next_id()}",
                            engine=inst.engine,
                            ins=[],
                            outs=[],
                            sync_info=mybir.SyncInfo(
                                on_wait=extra[j : j + _MAXW], on_update=[]
                            ),
                        )
                        nc.register_instruction(w_inst, overwrite=True)
                        new.append(w_inst)
                new.append(inst)
            b.instructions[:] = new

# ---------------------------------------------------------------------------
N_CORES = 8
C = 256
CH = 32            # channels per core
H = W = 112
NIMG = 8
PAD = 5
WP = W + 2 * PAD   # 122, horizontally padded row
VL = 240           # skew vector length (h + j spans [0, 238]; padded for M=128 FWL)
EPS = 1e-5
NHW = NIMG * H * W
NB = 6
CPC = 16           # channels per chunk
NCHUNK = CH // CPC
F32 = mybir.dt.float32
F16 = mybir.dt.float16

# (name, K, dilation)
BRANCHES = [("origin", 11, 1), ("k5_1", 5, 1), ("k7_1", 7, 1),
            ("k5_2", 5, 2), ("k3_3", 3, 3), ("k3_5", 3, 5)]

# mats: flat list of (branch_idx, dxoff) in branch order, kx ascending
MATS = []
for _bi, (_n, _K, _d) in enumerate(BRANCHES):
    _ctr = (_K - 1) // 2
    for _kx in range(_K):
        MATS.append((_bi, _d * (_kx - _ctr)))
NMAT1 = len(MATS)  # 34
BR_MATS = [[m for m, (bi, _) in enumerate(MATS) if bi == b] for b in range(NB)]


def _build_nc():
    nc = bass.Bass()
    xp = nc.declare_dram_parameter("xp", [H, CH, NIMG, WP], F16, isOutput=False)
    v1 = nc.declare_dram_parameter("v1", [CH, NMAT1, VL], F16, isOutput=False)
    gb = nc.declare_dram_parameter("gb", [2, CH, NB], F32, isOutput=False)
    outp = nc.declare_dram_parameter("outp", [H, CH, NIMG, W], F32, isOutput=True)
    sdram = nc.dram_tensor("s_scratch", [CH, NB], F32)
    tdram = nc.dram_tensor("t_scratch", [CH], F32)
    v2dram = nc.dram_tensor("v2_scratch", [CH, 11, VL], F16)

    with tile.TileContext(nc) as tc:
        spool = tc.alloc_tile_pool(name="small", bufs=1)
        xpool = tc.alloc_tile_pool(name="x", bufs=2)
        bpool = tc.alloc_tile_pool(name="bands", bufs=3)
        opool = tc.alloc_tile_pool(name="ob", bufs=2)
        ps1 = tc.alloc_tile_pool(name="ps1", bufs=2, space="PSUM")

        sy = spool.tile([H, NB * CH * 2], F32)    # sum(y) columns: c*12 + br*2 + half
        sq = spool.tile([H, NB * CH * 2], F32)    # sum(y^2) columns
        v1sb = spool.tile([CH, NMAT1, VL], F16)
        nc.sync.dma_start(out=v1sb[:], in_=v1[:])

        dma_engs = [nc.sync, nc.scalar, nc.gpsimd]

        # ---------------- pass 1: branch convs + raw stats ----------------
        def x_chunk(chunk):
            x_t = xpool.tile([H, CPC, NIMG, WP], F16, tag="x")
            nc.sync.dma_start(out=x_t[:], in_=xp[:, chunk * CPC:(chunk + 1) * CPC])
            return x_t

        x_tiles = [x_chunk(ch) for ch in range(NCHUNK)]
        for chunk in range(NCHUNK):
            x_t = x_tiles[chunk]
            for cl in range(CPC):
                c = chunk * CPC + cl
                b1 = bpool.tile([H, NMAT1, 128], F16, tag="bands")
                # split across two issuing engines -> more parallel DMA queues
                e0 = dma_engs[c % 3]
                e1 = dma_engs[(c + 1) % 3]
                hm = NMAT1 // 2
                e0.dma_start(
                    out=b1[:, 0:hm],
                    in_=bass.AP(tensor=v1, offset=c * NMAT1 * VL,
                                ap=[[1, H], [VL, hm], [1, 128]]),
                )
                e1.dma_start(
                    out=b1[:, hm:NMAT1],
                    in_=bass.AP(tensor=v1, offset=(c * NMAT1 + hm) * VL,
                                ap=[[1, H], [VL, NMAT1 - hm], [1, 128]]),
                )
                for br in range(NB):
                    mlist = BR_MATS[br]
                    py0 = ps1.tile([128, 4 * W], F32, tag="y0")
                    py1 = ps1.tile([128, 4 * W], F32, tag="y1")
                    for ki, m in enumerate(mlist):
                        dxo = MATS[m][1] + PAD
                        st = ki == 0
                        sp = ki == len(mlist) - 1
                        lhsT = b1[:, m]
                        nc.tensor.matmul(py0[:], lhsT, x_t[:, cl, 0:4, dxo:dxo + W],
                                         start=st, stop=sp)
                        nc.tensor.matmul(py1[:], lhsT, x_t[:, cl, 4:8, dxo:dxo + W],
                                         start=st, stop=sp)
                    col = (c * NB + br) * 2
                    nc.vector.tensor_reduce(out=sy[:, col:col + 1], in_=py0[:H],
                                            axis=mybir.AxisListType.X,
                                            op=mybir.AluOpType.add)
                    nc.vector.tensor_reduce(out=sy[:, col + 1:col + 2], in_=py1[:H],
                                            axis=mybir.AxisListType.X,
                                            op=mybir.AluOpType.add)
                    sq0 = ps1.tile([128, 4 * W], F32, tag="sqs")
                    nc.scalar.activation(out=sq0[:H], in_=py0[:H],
                                         func=mybir.ActivationFunctionType.Square,
                                         accum_out=sq[:, col:col + 1])
                    sq1 = ps1.tile([128, 4 * W], F32, tag="sqs")
                    nc.scalar.activation(out=sq1[:H], in_=py1[:H],
                                         func=mybir.ActivationFunctionType.Square,
                                         accum_out=sq[:, col + 1:col + 2])

        # ---------------- stats finalize (on partition 0) ----------------
        ones = spool.tile([H, 1], F32)
        nc.vector.memset(ones[:], 1.0)
        ps_sy = ps1.tile([1, NB * CH * 2], F32, tag="st")
        ps_sq = ps1.tile([1, NB * CH * 2], F32, tag="st")
        nc.tensor.matmul(ps_sy[:], ones[:], sy[:], start=True, stop=True)
        nc.tensor.matmul(ps_sq[:], ones[:], sq[:], start=True, stop=True)

        n192 = NB * CH
        Sy = spool.tile([1, n192], F32)
        Sq = spool.tile([1, n192], F32)
        nc.vector.tensor_reduce(
            out=Sy[:], in_=ps_sy[:].rearrange("p (a b) -> p a b", b=2),
            axis=mybir.AxisListType.X, op=mybir.AluOpType.add)
        nc.vector.tensor_reduce(
            out=Sq[:], in_=ps_sq[:].rearrange("p (a b) -> p a b", b=2),
            axis=mybir.AxisListType.X, op=mybir.AluOpType.add)

        m_t = spool.tile([1, n192], F32)
        nc.vector.tensor_scalar_mul(m_t[:], Sy[:], 1.0 / NHW)
        msq = spool.tile([1, n192], F32)
        nc.vector.tensor_mul(msq[:], m_t[:], m_t[:])
        v_t = spool.tile([1, n192], F32)
        nc.vector.scalar_tensor_tensor(
            out=v_t[:], in0=Sq[:], scalar=1.0 / NHW, in1=msq[:],
            op0=mybir.AluOpType.mult, op1=mybir.AluOpType.subtract)
        eps_t = spool.tile([1, 1], F32)
        nc.vector.memset(eps_t[:], EPS)
        std = spool.tile([1, n192], F32)
        nc.scalar.activation(out=std[:], in_=v_t[:],
                             func=mybir.ActivationFunctionType.Sqrt,
                             bias=eps_t[:], scale=1.0)
        r_t = spool.tile([1, n192], F32)
        nc.vector.reciprocal(r_t[:], std[:])

        gbsb = spool.tile([1, 2 * n192], F32)
        nc.sync.dma_start(out=gbsb[:],
                          in_=bass.AP(tensor=gb, offset=0, ap=[[0, 1], [1, 2 * n192]]))
        s_t = spool.tile([1, n192], F32)
        nc.vector.tensor_mul(s_t[:], r_t[:], gbsb[:, 0:n192])
        ms_t = spool.tile([1, n192], F32)
        nc.vector.tensor_mul(ms_t[:], m_t[:], s_t[:])
        t_t = spool.tile([1, n192], F32)
        nc.vector.scalar_tensor_tensor(
            out=t_t[:], in0=ms_t[:], scalar=-1.0, in1=gbsb[:, n192:2 * n192],
            op0=mybir.AluOpType.mult, op1=mybir.AluOpType.add)
        T_t = spool.tile([1, CH], F32)
        nc.vector.tensor_reduce(
            out=T_t[:], in_=t_t[:].rearrange("p (c b) -> p c b", b=NB),
            axis=mybir.AxisListType.X, op=mybir.AluOpType.add)
        # broadcast T to all 112 partitions via DRAM round-trip (stride-0 read)
        t_store = nc.sync.dma_start(
            out=bass.AP(tensor=tdram, offset=0, ap=[[0, 1], [1, CH]]), in_=T_t[:])
        T_b = spool.tile([H, CH], F32)
        t_load = nc.sync.dma_start(
            out=T_b[:], in_=bass.AP(tensor=tdram, offset=0, ap=[[0, H], [1, CH]]))
        tile.add_dep_helper(t_load.ins, t_store.ins, reason="T RAW via DRAM")

        # s -> [32 partitions, 6] via DRAM round-trip
        s_store = nc.sync.dma_start(
            out=bass.AP(tensor=sdram, offset=0, ap=[[0, 1], [NB, CH], [1, NB]]),
            in_=s_t[:].rearrange("p (c b) -> p c b", b=NB))
        s32 = spool.tile([CH, NB], F32)
        s_load = nc.sync.dma_start(out=s32[:], in_=sdram[:])
        tile.add_dep_helper(s_load.ins, s_store.ins, reason="s32 RAW via DRAM")

        # ---------------- merged kernel V2 = sum_br s_br * V1 ----------------
        v2sb = spool.tile([CH, 11, VL], F16)
        for m, (bi, dxoff) in enumerate(MATS):
            kxm = dxoff + PAD
            if bi == 0:
                nc.vector.tensor_scalar_mul(v2sb[:, kxm], v1sb[:, m], s32[:, 0:1])
            else:
                nc.vector.scalar_tensor_tensor(
                    out=v2sb[:, kxm], in0=v1sb[:, m], scalar=s32[:, bi:bi + 1],
                    in1=v2sb[:, kxm],
                    op0=mybir.AluOpType.mult, op1=mybir.AluOpType.add)
        v2_store = nc.sync.dma_start(out=v2dram[:], in_=v2sb[:])

        # ---------------- pass 2: merged conv + bias (reuses pass-1 x tiles) --
        for chunk in range(NCHUNK):
            x_t = x_tiles[chunk]
            for cl in range(CPC):
                c = chunk * CPC + cl
                b2 = bpool.tile([H, 11, 128], F16, tag="bands")
                b2_load = dma_engs[c % 3].dma_start(
                    out=b2[:],
                    in_=bass.AP(tensor=v2dram, offset=c * 11 * VL,
                                ap=[[1, H], [VL, 11], [1, 128]]),
                )
                tile.add_dep_helper(b2_load.ins, v2_store.ins, reason="v2 RAW via DRAM")
                po0 = ps1.tile([128, 4 * W], F32, tag="y0")
                po1 = ps1.tile([128, 4 * W], F32, tag="y1")
                for kxm in range(11):
                    dxo = kxm
                    st = kxm == 0
                    sp = kxm == 10
                    nc.tensor.matmul(po0[:], b2[:, kxm], x_t[:, cl, 0:4, dxo:dxo + W],
                                     start=st, stop=sp)
                    nc.tensor.matmul(po1[:], b2[:, kxm], x_t[:, cl, 4:8, dxo:dxo + W],
                                     start=st, stop=sp)
                ob = opool.tile([H, NIMG, W], F32, tag="ob")
                nc.vector.tensor_scalar_add(
                    ob[:, 0:4], po0[:H].rearrange("p (i w) -> p i w", w=W),
                    T_b[:, c:c + 1])
                nc.vector.tensor_scalar_add(
                    ob[:, 4:8], po1[:H].rearrange("p (i w) -> p i w", w=W),
                    T_b[:, c:c + 1])
                dma_engs[(c + 2) % 3].dma_start(out=outp[:, c], in_=ob[:])

        ps1.release()
        opool.release()
        bpool.release()
        xpool.release()
        spool.release()

    _split_excess_waits(nc)
    return nc


_NC_CACHE = {}


def _get_nc():
    if "nc" not in _NC_CACHE:
        _NC_CACHE["nc"] = _build_nc()
    return _NC_CACHE["nc"]


def _host_prep(inputs):
    x = np.asarray(inputs["x"], dtype=np.float32)
    in_maps = []
    for core in range(N_CORES):
        c0 = core * CH
        # xp[h, c, i, w] with flipped rows and horizontal zero padding
        xs = x[:, c0:c0 + CH]                       # [N, CH, H, W]
        xt = np.transpose(xs, (2, 1, 0, 3))[::-1]   # [H, CH, N, W], rows flipped
        xpb = np.zeros((H, CH, NIMG, WP), np.float16)
        xpb[:, :, :, PAD:PAD + W] = xt

        v1b = np.zeros((CH, NMAT1, VL), np.float16)
        m = 0
        for name, K, d in BRANCHES:
            wb = np.asarray(inputs[f"w_{name}"], dtype=np.float32)[c0:c0 + CH, 0]
            ctr = (K - 1) // 2
            for kx in range(K):
                for ky in range(K):
                    dy = d * (ky - ctr)
                    v1b[:, m, 111 - dy] = wb[:, ky, kx]
                m += 1

        gbb = np.zeros((2, CH, NB), np.float32)
        for bi, (name, K, d) in enumerate(BRANCHES):
            gbb[0, :, bi] = np.asarray(inputs[f"g_{name}"], dtype=np.float32)[c0:c0 + CH]
            gbb[1, :, bi] = np.asarray(inputs[f"b_{name}"], dtype=np.float32)[c0:c0 + CH]

        in_maps.append({"xp": np.ascontiguousarray(xpb),
                        "v1": v1b, "gb": gbb})
    return in_maps


def _get_runner():
    """Build (once) a cached sharded-jit executor for the Bass program.

    Mirrors concourse.bass2jax.run_bass_via_pjrt but (a) reuses the traced jit
    across calls and (b) creates the donated zero output buffers on-device
    instead of transferring ~100MB of host zeros per call."""
    if "runner" in _NC_CACHE:
        return _NC_CACHE["runner"]

    import jax
    import jax.numpy as jnp
    from jax.sharding import Mesh, PartitionSpec, NamedSharding
    from jax.experimental.shard_map import shard_map
    from concourse.bass2jax import (
        _bass_exec_p, install_neuronx_cc_hook, partition_id_tensor)

    install_neuronx_cc_hook()
    nc = _get_nc()
    part_name = nc.partition_id_tensor.name if nc.partition_id_tensor else None
    in_names, out_names, out_avals = [], [], []
    for alloc in nc.m.functions[0].allocations:
        if not isinstance(alloc, mybir.MemoryLocationSet):
            continue
        name = alloc.memorylocations[0].name
        if alloc.kind == "ExternalInput":
            if name != part_name:
                in_names.append(name)
        elif alloc.kind == "ExternalOutput":
            out_names.append(name)
            out_avals.append(jax.core.ShapedArray(
                tuple(alloc.tensor_shape), mybir.dt.np(alloc.dtype)))
    n_params = len(in_names)
    all_names = list(in_names) + list(out_names)
    if part_name is not None:
        all_names.append(part_name)

    def _body(*args):
        operands = list(args)
        if part_name is not None:
            operands.append(partition_id_tensor())
        outs = _bass_exec_p.bind(
            *operands,
            out_avals=tuple(out_avals),
            in_names=tuple(all_names),
            out_names=tuple(out_names),
            lowering_input_output_aliases=(),
            sim_require_finite=True,
            sim_require_nnan=True,
            nc=nc,
        )
        return tuple(outs)

    devices = jax.devices()[:N_CORES]
    mesh = Mesh(np.asarray(devices), ("core",))
    n_outs = len(out_names)
    donate = tuple(range(n_params, n_params + n_outs))
    sharded = jax.jit(
        shard_map(_body, mesh=mesh,
                  in_specs=(PartitionSpec("core"),) * (n_params + n_outs),
                  out_specs=(PartitionSpec("core"),) * n_outs,
                  check_rep=False),
        donate_argnums=donate, keep_unused=True)
    sh = NamedSharding(mesh, PartitionSpec("core"))
    zero_fn = jax.jit(
        lambda: tuple(
            jnp.zeros((N_CORES * a.shape[0], *a.shape[1:]), a.dtype)
            for a in out_avals),
        out_shardings=(sh,) * n_outs)

    def run(in_maps):
        concat_in = [
            np.concatenate([in_maps[c][n] for c in range(N_CORES)], axis=0)
            for n in in_names
        ]
        dev_in = [jax.device_put(a, sh) for a in concat_in]
        outs = sharded(*dev_in, *zero_fn())
        return {
            name: np.asarray(outs[i]).reshape(N_CORES, *out_avals[i].shape)
            for i, name in enumerate(out_names)
        }

    _NC_CACHE["runner"] = run
    return run


def _assemble(outp_all):
    out = np.empty((NIMG, C, H, W), np.float32)
    for core in range(N_CORES):
        o = outp_all[core]                          # [H, CH, NIMG, W]
        out[:, core * CH:(core + 1) * CH] = np.transpose(o, (2, 1, 0, 3))
    return out


def kernel(**inputs):
    in_maps = _host_prep(inputs)
    try:
        from concourse._compat import axon_active
        use_cached_pjrt = axon_active()
    except Exception:
        use_cached_pjrt = True
    if use_cached_pjrt:
        outs = _get_runner()(in_maps)
        outp_all = outs["outp"]
    else:
        from concourse.bass_utils import run_bass_kernel_spmd
        res = run_bass_kernel_spmd(
            _get_nc(), in_maps, core_ids=list(range(N_CORES)))
        outp_all = [res.results[c]["outp"] for c in range(N_CORES)]
    return _assemble(outp_all)

